# revision 10
# baseline (speedup 1.0000x reference)
"""Trainium2 Bass kernel for nn_ComplexNetCustomParam_89739046683234.

Computation (see reference):
    P_re = psi_r (x) psi_r + psi_i (x) psi_i            [10,10]
    P_im = psi_r (x) psi_i - psi_i (x) psi_r            [10,10]
    M_re[k,a] = sum_ij P_re[i,j] A_real[k,i,j,a] - P_im[i,j] A_imag[k,i,j,a]
    out[t,k]  = sum_a x[t,a] M_re[k,a]                  [500000, 2]

Strategy (data-parallel over t, 8 cores):
  - Host: pad t to 8*63488 rows, shard by rows, hand each core its shard
    pre-transposed as xT [100, R] (C-contiguous) so the device streams
    [100, F] tiles at full DMA efficiency with the contraction dim (a=100)
    on partitions.
  - Device preamble (per core, replicated): build P_re / -P_im as flattened
    [100,1] columns (outer products on PE + a tiny DRAM round-trip to move
    the 10x10 across partitions), then M_T = M_re^T [100, 2] via 4
    accumulated matmuls against A viewed as [2, 100, 100].
  - Device main loop: out^T[32, N] = M_T32.T @ xT[:, N-slice] matmuls,
    N=512, col-tiled 4-wide across PSUM partition groups {0,32,64,96} so
    each PSUM->SBUF copy retires 2048 outputs, then DMA out^T chunks.
  - Host: transpose each core's out^T back, concat, trim padding.
"""

import numpy as np

import concourse.bass as bass
import concourse.bacc as bacc
import concourse.mybir as mybir
from concourse.tile import TileContext
from concourse.bass_utils import run_bass_kernel_spmd

FP32 = mybir.dt.float32

N_CORES = 8
N_FEAT = 100
N_CLS = 2
PSI = 10
BATCH = 500000

NMM = 512                # moving free-dim per matmul (fp32 max)
GROUP = 4 * NMM          # 4 col-tiled matmuls per PSUM tile -> 2048 outputs
R_PER_CORE = 31 * GROUP  # 63488 rows per core (padded)
PIECES = (16384, 16384, 16384, 14336)  # DMA piece sizes (sum = 63488)


def _preamble(nc, cpool, ppre, psi_re, psi_im, a_re, a_im, p_scratch):
    """Compute M_T (widened to [100, 32], cols 0-1 real, rest zero)."""
    psi_sb = cpool.tile([1, 2 * PSI], FP32)
    nc.gpsimd.dma_start(out=psi_sb[0:1, 0:PSI], in_=psi_re[:])
    nc.gpsimd.dma_start(out=psi_sb[0:1, PSI : 2 * PSI], in_=psi_im[:])
    pr = psi_sb[0:1, 0:PSI]
    pi = psi_sb[0:1, PSI : 2 * PSI]
    npi_sb = cpool.tile([1, PSI], FP32)
    nc.scalar.mul(npi_sb[0:1, :], pi, -1.0)

    # Outer products, K=1 matmuls: out[i,j] = lhs[i] * rhs[j]
    psum_p = ppre.tile([PSI, 2 * PSI], FP32)
    # P_re = pr(x)pr + pi(x)pi
    nc.tensor.matmul(psum_p[:, 0:PSI], pr, pr, start=True, stop=False)
    nc.tensor.matmul(psum_p[:, 0:PSI], pi, pi, start=False, stop=True)
    # -P_im = pi(x)pr + pr(x)(-pi)
    nc.tensor.matmul(psum_p[:, PSI : 2 * PSI], pi, pr, start=True, stop=False)
    nc.tensor.matmul(
        psum_p[:, PSI : 2 * PSI], pr, npi_sb[0:1, :], start=False, stop=True
    )
    p_sb = cpool.tile([PSI, 2 * PSI], FP32)
    nc.vector.tensor_copy(p_sb[:, :], psum_p[:, :])
    # Round-trip through DRAM to flatten [10,10] -> [100,1] partitions.
    nc.gpsimd.dma_start(out=p_scratch[:, :], in_=p_sb[:, :])
    pre_flat = cpool.tile([PSI * PSI, 1], FP32)
    npim_flat = cpool.tile([PSI * PSI, 1], FP32)
    nc.gpsimd.dma_start(out=pre_flat[:, :], in_=p_scratch[:, 0:PSI])
    nc.gpsimd.dma_start(out=npim_flat[:, :], in_=p_scratch[:, PSI : 2 * PSI])

    # A tiles [100(ij), 100(a)] per class, contiguous loads.
    a_tiles = []
    for k in range(N_CLS):
        tr = cpool.tile([PSI * PSI, N_FEAT], FP32, tag=f"a_re{k}")
        nc.gpsimd.dma_start(out=tr[:, :], in_=a_re[k])
        ti = cpool.tile([PSI * PSI, N_FEAT], FP32, tag=f"a_im{k}")
        nc.gpsimd.dma_start(out=ti[:, :], in_=a_im[k])
        a_tiles.append((tr, ti))

    # M_T[a, k] = sum_ij A_real[k,ij,a]*P_re[ij] + A_imag[k,ij,a]*(-P_im[ij])
    psum_mt = ppre.tile([N_FEAT, N_CLS], FP32)
    for k in range(N_CLS):
        tr, ti = a_tiles[k]
        nc.tensor.matmul(
            psum_mt[:, k : k + 1], tr[:, :], pre_flat[:, :], start=True, stop=False
        )
        nc.tensor.matmul(
            psum_mt[:, k : k + 1], ti[:, :], npim_flat[:, :], start=False, stop=True
        )
    # Widen the stationary to 32 columns (M_T in cols 0-1, zeros in 2-31)
    # so each col-tiled matmul writes its full 32-partition PSUM group;
    # the single PSUM->SBUF copy per group then reads only initialized
    # data. LDW cost is negligible either way.
    mt_sb = cpool.tile([N_FEAT, 32], FP32)
    nc.vector.memset(mt_sb[:, :], 0.0)
    nc.vector.tensor_copy(mt_sb[:, 0:N_CLS], psum_mt[:, :])
    return mt_sb


def _main_pass(nc, pieces, xpool, spool, pout, xT, outT_v, mt_sb):
    off = 0
    g0 = 0  # absolute group index
    for F in pieces:
        ng = F // GROUP
        xt = xpool.tile([N_FEAT, F], FP32, tag="xt")
        nc.sync.dma_start(out=xt[:, :], in_=xT[:, off : off + F])
        stage = spool.tile([128, ng * NMM], FP32, tag="stage")
        for g in range(ng):
            ps = pout.tile([128, NMM], FP32, tag="ps")
            for j in range(4):
                nc.tensor.matmul(
                    ps[32 * j : 32 * j + 32, :],
                    mt_sb[:, :],
                    xt[:, g * GROUP + j * NMM : g * GROUP + (j + 1) * NMM],
                    start=True,
                    stop=True,
                    tile_position=(0, 32 * j),
                )
            # One copy retires all 4 col-groups (2048 outputs).
            dst = stage[:, g * NMM : (g + 1) * NMM]
            if g % 2 == 0:
                nc.vector.tensor_copy(dst, ps[:, :])
            else:
                nc.scalar.copy(dst, ps[:, :])
        for j in range(4):
            nc.sync.dma_start(
                out=outT_v[:, g0 : g0 + ng, j, :],
                in_=stage[32 * j : 32 * j + N_CLS, :],
            )
        off += F
        g0 += ng


def build_nc(pieces=PIECES, reps=1):
    """Build the per-core Bass program (SPMD: all cores run this).

    reps > 1 repeats the main loop (same data) for wall-clock benchmarking
    via differencing; the preamble runs once.
    """
    R = sum(pieces)
    assert R % GROUP == 0
    nc = bacc.Bacc(None, target_bir_lowering=False, debug=False)

    xT = nc.declare_dram_parameter("xT", [N_FEAT, R], FP32, isOutput=False)
    a_re = nc.declare_dram_parameter(
        "A_real", [N_CLS, PSI * PSI, N_FEAT], FP32, isOutput=False
    )
    a_im = nc.declare_dram_parameter(
        "A_imag", [N_CLS, PSI * PSI, N_FEAT], FP32, isOutput=False
    )
    psi_re = nc.declare_dram_parameter("psi_real", [PSI], FP32, isOutput=False)
    psi_im = nc.declare_dram_parameter("psi_imag", [PSI], FP32, isOutput=False)
    outT = nc.declare_dram_parameter("outT", [N_CLS, R], FP32, isOutput=True)

    # Scratch for moving the 10x10 P matrices across partitions ([10,10] ->
    # flattened [100,1]); layout [10, 2*PSI] = [P_re row | -P_im row].
    p_scratch = nc.dram_tensor("p_scratch", [PSI, 2 * PSI], FP32)

    with TileContext(nc) as tc:
        with (
            tc.tile_pool(name="const", bufs=1) as cpool,
            tc.tile_pool(name="xin", bufs=2) as xpool,
            tc.tile_pool(name="stage", bufs=2) as spool,
            tc.tile_pool(name="psum_pre", bufs=2, space="PSUM") as ppre,
            tc.tile_pool(name="psum_out", bufs=4, space="PSUM") as pout,
        ):
            mt_sb = _preamble(nc, cpool, ppre, psi_re, psi_im, a_re, a_im, p_scratch)
            # outT viewed [k, group, colgrp, n] for the staged output DMAs.
            outT_v = outT.rearrange("k (c j n) -> k c j n", j=4, n=NMM)
            for _rep in range(reps):
                _main_pass(nc, pieces, xpool, spool, pout, xT, outT_v, mt_sb)

    nc.finalize()
    return nc


_NC_CACHE = {}


def _get_nc(reps=1):
    key = reps
    if key not in _NC_CACHE:
        _NC_CACHE[key] = build_nc(reps=reps)
    return _NC_CACHE[key]


def _shard_inputs(x, A_real, A_imag, psi_real, psi_imag):
    x = np.ascontiguousarray(np.asarray(x, dtype=np.float32))
    a_re = np.ascontiguousarray(
        np.asarray(A_real, dtype=np.float32).reshape(N_CLS, PSI * PSI, N_FEAT)
    )
    a_im = np.ascontiguousarray(
        np.asarray(A_imag, dtype=np.float32).reshape(N_CLS, PSI * PSI, N_FEAT)
    )
    psi_re = np.ascontiguousarray(np.asarray(psi_real, dtype=np.float32))
    psi_im = np.ascontiguousarray(np.asarray(psi_imag, dtype=np.float32))

    n_rows = x.shape[0]
    in_maps = []
    for c in range(N_CORES):
        s = c * R_PER_CORE
        e = min(s + R_PER_CORE, n_rows)
        if e - s == R_PER_CORE:
            shard_t = np.ascontiguousarray(x[s:e].T)
        else:
            shard_t = np.zeros((N_FEAT, R_PER_CORE), dtype=np.float32)
            if e > s:
                shard_t[:, : e - s] = x[s:e].T
        in_maps.append(
            {
                "xT": shard_t,
                "A_real": a_re,
                "A_imag": a_im,
                "psi_real": psi_re,
                "psi_imag": psi_im,
            }
        )
    return in_maps, n_rows


def kernel(x, A_real, A_imag, psi_real, psi_imag):
    in_maps, n_rows = _shard_inputs(x, A_real, A_imag, psi_real, psi_imag)
    res = run_bass_kernel_spmd(_get_nc(), in_maps, core_ids=list(range(N_CORES)))
    out = np.concatenate([r["outT"].T for r in res.results], axis=0)
    return np.ascontiguousarray(out[:n_rows])


# revision 22
# speedup vs baseline: 1.0450x; 1.0450x over previous
"""Trainium2 Bass kernel for nn_ComplexNetCustomParam_89739046683234.

Computation (see reference):
    P_re = psi_r (x) psi_r + psi_i (x) psi_i            [10,10]
    P_im = psi_r (x) psi_i - psi_i (x) psi_r            [10,10]
    M_re[k,a] = sum_ij P_re[i,j] A_real[k,i,j,a] - P_im[i,j] A_imag[k,i,j,a]
    out[t,k]  = sum_a x[t,a] M_re[k,a]                  [500000, 2]

Strategy (data-parallel over t, 8 cores):
  - Host: pad t to 8*63488 rows, shard by rows, hand each core its shard
    pre-transposed as xT [100, R] (C-contiguous) so the device streams
    [100, F] tiles at full DMA efficiency with the contraction dim (a=100)
    on partitions.
  - Device preamble (per core, replicated): build P_re / -P_im as flattened
    [100,1] columns (outer products on PE + a tiny DRAM round-trip to move
    the 10x10 across partitions), then M_T = M_re^T [100, 2] via 4
    accumulated matmuls against A viewed as [2, 100, 100].
  - Device main loop: out^T[32, N] = M_T32.T @ xT[:, N-slice] matmuls,
    N=512, col-tiled 4-wide across PSUM partition groups {0,32,64,96} so
    each PSUM->SBUF copy retires 2048 outputs, then DMA out^T chunks.
  - Host: transpose each core's out^T back, concat, trim padding.
"""

import numpy as np

import concourse.bass as bass
import concourse.bacc as bacc
import concourse.mybir as mybir
from concourse.tile import TileContext
from concourse.bass_utils import run_bass_kernel_spmd

FP32 = mybir.dt.float32

N_CORES = 8
N_FEAT = 100
N_CLS = 2
PSI = 10
BATCH = 500000

NMM = 512                # moving free-dim per matmul (fp32 max)
GROUP = 4 * NMM          # 4 col-tiled matmuls per PSUM tile -> 2048 outputs
R_PER_CORE = 31 * GROUP  # 63488 rows per core (padded)
PIECES = (4096,) * 15 + (2048,)  # DMA piece sizes (sum = 63488)


def _preamble(nc, cpool, ppre, psi_re, psi_im, a_re, a_im, p_scratch):
    """Compute M_T (widened to [100, 32], cols 0-1 real, rest zero)."""
    psi_sb = cpool.tile([1, 2 * PSI], FP32)
    nc.gpsimd.dma_start(out=psi_sb[0:1, 0:PSI], in_=psi_re[:])
    nc.gpsimd.dma_start(out=psi_sb[0:1, PSI : 2 * PSI], in_=psi_im[:])
    pr = psi_sb[0:1, 0:PSI]
    pi = psi_sb[0:1, PSI : 2 * PSI]
    npi_sb = cpool.tile([1, PSI], FP32)
    nc.scalar.mul(npi_sb[0:1, :], pi, -1.0)

    # Outer products, K=1 matmuls: out[i,j] = lhs[i] * rhs[j]
    psum_p = ppre.tile([PSI, 2 * PSI], FP32)
    # P_re = pr(x)pr + pi(x)pi
    nc.tensor.matmul(psum_p[:, 0:PSI], pr, pr, start=True, stop=False)
    nc.tensor.matmul(psum_p[:, 0:PSI], pi, pi, start=False, stop=True)
    # -P_im = pi(x)pr + pr(x)(-pi)
    nc.tensor.matmul(psum_p[:, PSI : 2 * PSI], pi, pr, start=True, stop=False)
    nc.tensor.matmul(
        psum_p[:, PSI : 2 * PSI], pr, npi_sb[0:1, :], start=False, stop=True
    )
    p_sb = cpool.tile([PSI, 2 * PSI], FP32)
    nc.vector.tensor_copy(p_sb[:, :], psum_p[:, :])
    # Round-trip through DRAM to flatten [10,10] -> [100,1] partitions.
    nc.gpsimd.dma_start(out=p_scratch[:, :], in_=p_sb[:, :])
    pre_flat = cpool.tile([PSI * PSI, 1], FP32)
    npim_flat = cpool.tile([PSI * PSI, 1], FP32)
    nc.gpsimd.dma_start(out=pre_flat[:, :], in_=p_scratch[:, 0:PSI])
    nc.gpsimd.dma_start(out=npim_flat[:, :], in_=p_scratch[:, PSI : 2 * PSI])

    # A tiles [100(ij), 100(a)] per class, contiguous loads.
    a_tiles = []
    for k in range(N_CLS):
        tr = cpool.tile([PSI * PSI, N_FEAT], FP32, tag=f"a_re{k}")
        nc.gpsimd.dma_start(out=tr[:, :], in_=a_re[k])
        ti = cpool.tile([PSI * PSI, N_FEAT], FP32, tag=f"a_im{k}")
        nc.gpsimd.dma_start(out=ti[:, :], in_=a_im[k])
        a_tiles.append((tr, ti))

    # M_T[a, k] = sum_ij A_real[k,ij,a]*P_re[ij] + A_imag[k,ij,a]*(-P_im[ij])
    psum_mt = ppre.tile([N_FEAT, N_CLS], FP32)
    for k in range(N_CLS):
        tr, ti = a_tiles[k]
        nc.tensor.matmul(
            psum_mt[:, k : k + 1], tr[:, :], pre_flat[:, :], start=True, stop=False
        )
        nc.tensor.matmul(
            psum_mt[:, k : k + 1], ti[:, :], npim_flat[:, :], start=False, stop=True
        )
    # Widen the stationary to 32 columns (M_T in cols 0-1, zeros in 2-31)
    # so each col-tiled matmul writes its full 32-partition PSUM group;
    # the single PSUM->SBUF copy per group then reads only initialized
    # data. LDW cost is negligible either way.
    mt_sb = cpool.tile([128, 32], FP32)
    nc.vector.memset(mt_sb[:, :], 0.0)
    nc.vector.tensor_copy(mt_sb[0:N_FEAT, 0:N_CLS], psum_mt[:, :])
    return mt_sb


def _main_pass(nc, pieces, xpool, spool, pout, xT, outT_v, mt_sb,
               rings="sp", mode="full", out_ring="act", pre_xt=None,
               layout="rowmajor"):
    ring_map = {"sp": [nc.sync], "act": [nc.scalar], "alt": [nc.sync, nc.scalar],
                "pool": [nc.gpsimd],
                "alt3": [nc.sync, nc.scalar, nc.gpsimd]}
    in_engines = ring_map[rings]
    out_eng = {"act": nc.scalar, "sp": nc.sync, "pool": nc.gpsimd}[out_ring]
    off = 0
    g0 = 0  # absolute group index
    for pi_, F in enumerate(pieces):
        ng = F // GROUP
        if mode == "compute_only":
            xt = pre_xt[pi_]
        elif mode == "dma128":
            w = F * N_FEAT // 128
            xt = xpool.tile([128, w], FP32, tag="xt")
            flat = xT.rearrange("a r -> (a r)")
            seg = flat[off * N_FEAT : off * N_FEAT + F * N_FEAT]
            in_engines[pi_ % len(in_engines)].dma_start(
                out=xt[:, :], in_=seg.rearrange("(p w) -> p w", p=128)
            )
        elif layout == "pieced":
            xt = xpool.tile([N_FEAT, F], FP32, tag="xt")
            soff = N_FEAT * off
            seg = xT[soff : soff + N_FEAT * F]
            in_engines[pi_ % len(in_engines)].dma_start(
                out=xt[:, :], in_=seg.rearrange("(a f) -> a f", a=N_FEAT)
            )
        else:
            npart = 128 if layout == "pad128" else N_FEAT
            xt = xpool.tile([npart, F], FP32, tag="xt")
            in_engines[pi_ % len(in_engines)].dma_start(
                out=xt[:, :], in_=xT[:, off : off + F]
            )
        stage = spool.tile([128, ng * NMM], FP32, tag="stage")
        if mode in ("dma_only", "dma128"):
            # Tiny consumer so the loads aren't dead: copy one column out.
            nc.vector.tensor_copy(
                stage[0:N_FEAT, pi_ : pi_ + 1], xt[0:N_FEAT, 0:1]
            )
            if pi_ == len(pieces) - 1:
                out_eng.dma_start(
                    out=outT_v[0:1, 0:1, 0, 0 : len(pieces)],
                    in_=stage[0:1, 0 : len(pieces)],
                )
            off += F
            g0 += ng
            continue
        for g in range(ng):
            ps = pout.tile([128, NMM], FP32, tag="ps")
            kdim = xt.shape[0]
            for j in range(4):
                nc.tensor.matmul(
                    ps[32 * j : 32 * j + 32, :],
                    mt_sb[0:kdim, :],
                    xt[:, g * GROUP + j * NMM : g * GROUP + (j + 1) * NMM],
                    start=True,
                    stop=True,
                    tile_position=(0, 32 * j),
                )
            # One copy retires all 4 col-groups (2048 outputs).
            dst = stage[:, g * NMM : (g + 1) * NMM]
            if g % 2 == 0:
                nc.vector.tensor_copy(dst, ps[:, :])
            else:
                nc.scalar.copy(dst, ps[:, :])
        for j in range(4):
            out_eng.dma_start(
                out=outT_v[:, g0 : g0 + ng, j, :],
                in_=stage[32 * j : 32 * j + N_CLS, :],
            )
        off += F
        g0 += ng


def build_nc(pieces=PIECES, reps=1, rings="sp", mode="full", out_ring="act",
             layout="pad128", xbufs=None, sbufs=None):
    """Build the per-core Bass program (SPMD: all cores run this).

    reps > 1 repeats the main loop (same data) for wall-clock benchmarking
    via differencing; the preamble runs once.
    """
    R = sum(pieces)
    assert R % GROUP == 0
    nc = bacc.Bacc(None, target_bir_lowering=False, debug=False)

    if layout == "pieced":
        xT = nc.declare_dram_parameter("xT", [N_FEAT * R], FP32, isOutput=False)
    elif layout == "pad128":
        xT = nc.declare_dram_parameter("xT", [128, R], FP32, isOutput=False)
    else:
        xT = nc.declare_dram_parameter("xT", [N_FEAT, R], FP32, isOutput=False)
    a_re = nc.declare_dram_parameter(
        "A_real", [N_CLS, PSI * PSI, N_FEAT], FP32, isOutput=False
    )
    a_im = nc.declare_dram_parameter(
        "A_imag", [N_CLS, PSI * PSI, N_FEAT], FP32, isOutput=False
    )
    psi_re = nc.declare_dram_parameter("psi_real", [PSI], FP32, isOutput=False)
    psi_im = nc.declare_dram_parameter("psi_imag", [PSI], FP32, isOutput=False)
    outT = nc.declare_dram_parameter("outT", [N_CLS, R], FP32, isOutput=True)

    # Scratch for moving the 10x10 P matrices across partitions ([10,10] ->
    # flattened [100,1]); layout [10, 2*PSI] = [P_re row | -P_im row].
    p_scratch = nc.dram_tensor("p_scratch", [PSI, 2 * PSI], FP32)

    with TileContext(nc) as tc:
        mp = max(pieces)
        if xbufs is None:
            xbufs = 2 if mp > 8192 else (4 if mp >= 4096 else 8)
        if sbufs is None:
            sbufs = 2 if mp > 4096 else 4
        with (
            tc.tile_pool(name="const", bufs=1) as cpool,
            tc.tile_pool(name="xin", bufs=xbufs) as xpool,
            tc.tile_pool(name="stage", bufs=sbufs) as spool,
            tc.tile_pool(name="psum_pre", bufs=1, space="PSUM") as ppre,
            tc.tile_pool(name="psum_out", bufs=6, space="PSUM") as pout,
        ):
            mt_sb = _preamble(nc, cpool, ppre, psi_re, psi_im, a_re, a_im, p_scratch)
            # outT viewed [k, group, colgrp, n] for the staged output DMAs.
            outT_v = outT.rearrange("k (c j n) -> k c j n", j=4, n=NMM)
            pre_xt = None
            if mode == "compute_only":
                pre_xt = []
                for pi_, F in enumerate(pieces):
                    xt = xpool.tile([N_FEAT, F], FP32, tag=f"xt{pi_}")
                    nc.sync.dma_start(
                        out=xt[:, :], in_=xT[:, sum(pieces[:pi_]) : sum(pieces[:pi_]) + F]
                    )
                    pre_xt.append(xt)
            for _rep in range(reps):
                _main_pass(nc, pieces, xpool, spool, pout, xT, outT_v, mt_sb,
                           rings=rings, mode=mode, out_ring=out_ring,
                           pre_xt=pre_xt, layout=layout)

    nc.finalize()
    return nc


_NC_CACHE = {}


def _get_nc(reps=1, pieces=PIECES, **kw):
    key = (reps, tuple(pieces), tuple(sorted(kw.items())))
    if key not in _NC_CACHE:
        _NC_CACHE[key] = build_nc(pieces=pieces, reps=reps, **kw)
    return _NC_CACHE[key]


def _shard_inputs(x, A_real, A_imag, psi_real, psi_imag, layout="pad128",
                  pieces=PIECES):
    x = np.ascontiguousarray(np.asarray(x, dtype=np.float32))
    a_re = np.ascontiguousarray(
        np.asarray(A_real, dtype=np.float32).reshape(N_CLS, PSI * PSI, N_FEAT)
    )
    a_im = np.ascontiguousarray(
        np.asarray(A_imag, dtype=np.float32).reshape(N_CLS, PSI * PSI, N_FEAT)
    )
    psi_re = np.ascontiguousarray(np.asarray(psi_real, dtype=np.float32))
    psi_im = np.ascontiguousarray(np.asarray(psi_imag, dtype=np.float32))

    n_rows = x.shape[0]
    in_maps = []
    for c in range(N_CORES):
        s = c * R_PER_CORE
        e = min(s + R_PER_CORE, n_rows)
        if e - s == R_PER_CORE:
            shard_t = np.ascontiguousarray(x[s:e].T)
        else:
            shard_t = np.zeros((N_FEAT, R_PER_CORE), dtype=np.float32)
            if e > s:
                shard_t[:, : e - s] = x[s:e].T
        if layout == "pieced":
            segs = []
            off = 0
            for F in pieces:
                segs.append(shard_t[:, off : off + F].ravel())
                off += F
            shard_t = np.concatenate(segs)
        elif layout == "pad128":
            pad = np.zeros((128, R_PER_CORE), dtype=np.float32)
            pad[:N_FEAT] = shard_t
            shard_t = pad
        in_maps.append(
            {
                "xT": shard_t,
                "A_real": a_re,
                "A_imag": a_im,
                "psi_real": psi_re,
                "psi_imag": psi_im,
            }
        )
    return in_maps, n_rows


def kernel(x, A_real, A_imag, psi_real, psi_imag):
    in_maps, n_rows = _shard_inputs(x, A_real, A_imag, psi_real, psi_imag)
    res = run_bass_kernel_spmd(_get_nc(), in_maps, core_ids=list(range(N_CORES)))
    out = np.concatenate([r["outT"].T for r in res.results], axis=0)
    return np.ascontiguousarray(out[:n_rows])
